# revision 39
# baseline (speedup 1.0000x reference)
"""Trainium2 Bass kernel for nn_DenseAttentionOneHead (B=2, L=4096, H=1024).

Reference math:
    h   = hidden * cos + rotate_half(hidden) * sin      (RoPE)
    q   = h @ W_q.T
    out = (q @ h^T) @ h                                 (no softmax)

With no softmax the L x L score matrix factorizes away, and W_q folds in:
    out[b] = h[b] @ M[b],  M[b] = W_q^T G[b],  G[b] = h[b].T @ h[b]  (H x H)

Sharding (8 NeuronCores): cores 0-3 own batch 0's four 1024-row L-chunks,
cores 4-7 batch 1. Each core computes a partial G over its chunk. Instead
of a 4MB fp32 AllReduce of G (cost-model ~225us), the cross-core exchange
is a bf16 ReduceScatter of G (0.5MB out, ~28us) -> each core computes its
256-column slice of M = W_q^T G -> bf16 AllGather of M (2MB out, ~67us).
G is symmetric, so a row-shard of G is a column-shard: the bounce buffers
are laid out shard-major ([rank][k][col-within-shard]) so both the
RS output and the AG output arrive in exactly the layout the next matmul
wants -- no transposes or reshuffles on the critical path.

Engine plan per core: DVE does RoPE tiles 0-6 (+ W/M dequants), Pool does
RoPE tile 7; PE pipelines G's first column-half into 8 persistent PSUM
accumulators while RoPE streams, drains them interleaved with the second
column-half pass, does the h^T transposes (4 per PSUM bank, one wide copy)
under the RS, then the M-slice and y matmuls. No-reader self-matmuls keep
the PE p-state ramped through both collective windows (the cost model
prices post-idle matmuls up to 3.7x slower). ACT/DVE split the PSUM->SBUF
copies; the two HWDGE rings split the traffic (h/G-out/M-in-write/y-even
on sync, cos/sin/W/G-slice/M-fetch/y-odd on scalar).
"""

import os

import numpy as np

import jax

try:
    _cache_dir = os.path.join(os.path.expanduser("~"), ".cache", "bass_kernel_jax")
    os.makedirs(_cache_dir, exist_ok=True)
    jax.config.update("jax_compilation_cache_dir", _cache_dir)
    jax.config.update("jax_persistent_cache_min_compile_time_secs", 1.0)
except Exception:
    pass

import concourse.bacc as bacc
import concourse.mybir as mybir
import concourse.tile as tile
from concourse import masks
from concourse.bass_utils import run_bass_kernel_spmd

F32 = mybir.dt.float32
F32R = mybir.dt.float32r
BF16 = mybir.dt.bfloat16

B, L, H = 2, 4096, 1024
L_CHUNK = 1024
HH = H // 2
NT = L_CHUNK // 128   # 8 row tiles per chunk
MT = H // 128         # 8 column tiles
NSH = H // 4          # 256: columns per RS shard
GROUPS = [[0, 1, 2, 3], [4, 5, 6, 7]]
NACC = 8              # PSUM bank ring ("ps" tag)
WARM1 = 124           # PE keep-warm matmuls during the ReduceScatter
WARM2 = 359           # PE keep-warm matmuls during the AllGather


def _emit_once(nc, tc, h_d, c1_d, s1_d, wq_d, y_d):
    h_ap = h_d.ap().rearrange("(t p) c -> p t c", p=128)
    c1_ap = c1_d.ap().rearrange("(t p) c -> p t c", p=128)
    s1_ap = s1_d.ap().rearrange("(t p) c -> p t c", p=128)
    wq_ap = wq_d.ap().rearrange("(t p) c -> p t c", p=128)
    y_ap = y_d.ap().rearrange("(t p) c -> p t c", p=128)

    with (
        tc.tile_pool(name="persist", bufs=1) as persist,
        tc.tile_pool(name="stream", bufs=1) as stream,
        tc.tile_pool(name="pacc", bufs=NACC, space="PSUM") as pacc,
        tc.tile_pool(name="dram", bufs=1, space="DRAM") as dram,
    ):
        hr = persist.tile([128, NT, H], F32R, name="hr")
        hrt = persist.tile([128, MT, L_CHUNK], F32R, name="hrt")
        wq_b = persist.tile([128, MT, H], BF16, name="wq_b")
        mr = persist.tile([128, MT, H], F32R, name="mr")
        gsl = persist.tile([128, MT, NSH], BF16, name="gsl")

        # DRAM bounce buffers, shard-major layout:
        #   g_in  flat = rank*262144 + k*256 + nw   <->  G_part[k, 256*rank+nw]
        #   g_rs  flat = k*256 + nw                 <->  G[k, 256*myrank+nw]
        #   m_in  flat = hcol*256 + nw              <->  M[hcol, 256*myrank+nw]
        #   m_out flat = rank*262144 + h*256 + nw   <->  M[h, 256*rank+nw]
        g_in = dram.tile([128, MT * H], BF16, name="g_in")
        g_rs = dram.tile([32, MT * H], BF16, name="g_rs")
        m_in = dram.tile([32, MT * H], BF16, name="m_in")
        m_out = dram.tile([128, MT * H], BF16, name="m_out")
        gv = g_in[:].rearrange("(r kp) (kq nw) -> kp kq r nw", r=4, kq=32)
        # one-DMA fetch views: [pp, kt, ...] with k = kt*128 + pp
        rv = (
            g_rs[:]
            .rearrange("p (kq nw) -> (p kq) nw", kq=32)
            .rearrange("(kt pp) nw -> pp kt nw", pp=128)
        )
        mv = (
            m_in[:]
            .rearrange("p (kq nw) -> (p kq) nw", kq=32)
            .rearrange("(mt pp) nw -> pp mt nw", pp=128)
        )
        ov = m_out[:].rearrange("(r kp) (kq nw) -> kp kq r nw", r=4, kq=32)

        # persistent PSUM accumulators for G[:, 0:512], all 8 row bands
        gacc = [
            pacc.tile([128, 512], F32, name=f"gacc{i}", tag="ps")
            for i in range(MT)
        ]

        # ---- RoPE + pipelined G first-half. Tiles 0-6 stream on DVE;
        # tile 7 runs on the otherwise-idle Pool engine so the two engines
        # finish together. ----
        for t in range(NT):
            ht = stream.tile([128, H], F32, name="ht", tag="ld1024", bufs=5)
            ct = stream.tile([128, HH], F32, name="ct", tag="ldc", bufs=5)
            st = stream.tile([128, HH], F32, name="st", tag="lds", bufs=5)
            nc.sync.dma_start(ht[:], h_ap[:, t, :])
            nc.scalar.dma_start(ct[:], c1_ap[:, t, :])
            nc.scalar.dma_start(st[:], s1_ap[:, t, :])
            h1 = ht[:, 0:HH]
            h2 = ht[:, HH:H]
            tag = "tmp" if t < 7 else "tmpp"
            m1 = stream.tile([128, HH], F32, name="m1", tag=tag, bufs=4)
            m2 = stream.tile([128, HH], F32, name="m2", tag=tag, bufs=4)
            m3 = stream.tile([128, HH], F32, name="m3", tag=tag, bufs=4)
            m4 = stream.tile([128, HH], F32, name="m4", tag=tag, bufs=4)
            eng = nc.vector if t < 7 else nc.gpsimd
            eng.tensor_mul(m1[:], h1, ct[:])
            eng.tensor_mul(m2[:], h2, st[:])
            eng.tensor_sub(hr[:, t, 0:HH], m1[:], m2[:])
            eng.tensor_mul(m3[:], h2, ct[:])
            eng.tensor_mul(m4[:], h1, st[:])
            eng.tensor_add(hr[:, t, HH:H], m3[:], m4[:])
            # G[:, 0:512] accumulation rides the RoPE stream
            for mt in range(MT):
                nc.tensor.matmul(
                    gacc[mt][:],
                    hr[:, t, mt * 128:(mt + 1) * 128],
                    hr[:, t, 0:512],
                    start=(t == 0),
                    stop=(t == NT - 1),
                )

        ident_f = stream.tile([128, 128], F32, name="ident_f", tag="identf")
        masks.make_identity(nc, ident_f[:])
        ident = stream.tile([128, 128], F32R, name="ident", tag="ident")
        nc.vector.tensor_copy(ident[:], ident_f[:])

        # W_q load (scalar ring, behind c/s) + bf16 pack on DVE (idle
        # post-RoPE; keeps the ACT queue clear for the G pack copies)
        for mt in range(MT):
            wt = stream.tile([128, H], F32, name="wt", tag="ldw", bufs=2)
            nc.scalar.dma_start(wt[:], wq_ap[:, mt, :])
            nc.vector.tensor_copy(wq_b[:, mt, :], wt[:])

        # drain the pipelined accumulators while computing G cols 512:1024;
        # both halves of a row band pack into one [128,1024] tile -> one DMA
        # (the shard-major dst AP merges to 3 dims for full-width rows).
        for mt in range(MT):
            gb2 = stream.tile([128, H], BF16, name="gb2", tag="gb", bufs=3)
            nc.scalar.copy(gb2[:, 0:512], gacc[mt][:])
            ps = pacc.tile([128, 512], F32, name="ps1", tag="ps")
            for kt in range(NT):
                nc.tensor.matmul(
                    ps[:],
                    hr[:, kt, mt * 128:(mt + 1) * 128],
                    hr[:, kt, 512:1024],
                    start=(kt == 0),
                    stop=(kt == NT - 1),
                )
            nc.scalar.copy(gb2[:, 512:1024], ps[:])
            nc.sync.dma_start(gv[4 * mt:4 * (mt + 1), :, :, :], gb2[:])

        # ReduceScatter of partial G (bf16): each core gets its 256 columns
        nc.gpsimd.collective_compute(
            "ReduceScatter",
            mybir.AluOpType.add,
            replica_groups=GROUPS,
            ins=[g_in[:]],
            outs=[g_rs[:]],
        )

        # transposes for the y-phase stationary h^T (PE idles during RS).
        # 4 transposes share one PSUM bank + one wide ACT copy, so the
        # PE<->ACT semaphore round-trip amortizes 4x.
        for mt in range(MT):
            for ta in range(0, NT, 4):
                pstb = pacc.tile([128, 512], F32R, name="pstb", tag="ps")
                for j in range(4):
                    nc.tensor.transpose(
                        pstb[:, j * 128:(j + 1) * 128],
                        hr[:, ta + j, mt * 128:(mt + 1) * 128],
                        ident[:],
                    )
                nc.vector.tensor_copy(
                    hrt[:, mt, ta * 128:(ta + 4) * 128], pstb[:]
                )

        # self-matmuls with no readers keep the PE p-state ramped through the
        # RS wait (cost model prices post-idle matmuls up to 3.7x slower)
        js = pacc.tile([128, 512], F32, name="js", tag="ps")
        for _ in range(WARM1):
            nc.tensor.matmul(
                js[:], hr[:, 0, 0:128], hr[:, 0, 0:512],
                start=True, stop=True, skip_group_check=True,
            )

        # fetch my G column slice (already [k, nw] matmul layout), one DMA
        nc.scalar.dma_start(gsl[:, :, :], rv[:, :, :])

        # M slice = W_q^T G[:, mycols]  (bf16 matmuls, fp32 PSUM)
        mqall = persist.tile([128, MT, NSH], BF16, name="mqall")
        for mt in range(MT):
            ps = pacc.tile([128, NSH], F32, name="psm", tag="ps")
            for dk in range(MT):
                nc.tensor.matmul(
                    ps[:],
                    wq_b[:, dk, mt * 128:(mt + 1) * 128],
                    gsl[:, dk, :],
                    start=(dk == 0),
                    stop=(dk == MT - 1),
                )
            nc.scalar.copy(mqall[:, mt, :], ps[:])
        nc.sync.dma_start(mv[:, :, :], mqall[:, :, :])

        # AllGather of M slices (bf16)
        nc.gpsimd.collective_compute(
            "AllGather",
            mybir.AluOpType.bypass,
            replica_groups=GROUPS,
            ins=[m_in[:]],
            outs=[m_out[:]],
        )

        # keep the PE warm through the AllGather idle window
        js2 = pacc.tile([128, 512], F32, name="js2", tag="ps")
        for _ in range(WARM2):
            nc.tensor.matmul(
                js2[:], hr[:, 0, 0:128], hr[:, 0, 0:512],
                start=True, stop=True, skip_group_check=True,
            )

        # fetch + dequant M per-tile (DVE), pipelined with the y matmuls
        for ht in range(MT):
            mf = stream.tile([128, H], BF16, name="mf", tag="mf", bufs=3)
            nc.scalar.dma_start(mf[:], ov[4 * ht:4 * (ht + 1), :, :, :])
            nc.vector.tensor_copy(mr[:, ht, :], mf[:])

        # y = h @ M ; k-outer accumulation over all 8 row tiles at once
        for nh2 in range(2):
            psy = [
                pacc.tile([128, 512], F32, name=f"psy{lt}", tag="ps")
                for lt in range(NT)
            ]
            for ht in range(MT):
                for lt in range(NT):
                    nc.tensor.matmul(
                        psy[lt][:],
                        hrt[:, ht, lt * 128:(lt + 1) * 128],
                        mr[:, ht, nh2 * 512:(nh2 + 1) * 512],
                        start=(ht == 0),
                        stop=(ht == MT - 1),
                    )
            for lt in range(NT):
                yo = stream.tile([128, 512], F32, name="yo", tag="yo", bufs=4)
                if lt % 2 == 0:
                    nc.scalar.copy(yo[:], psy[lt][:])
                    nc.sync.dma_start(
                        y_ap[:, lt, nh2 * 512:(nh2 + 1) * 512], yo[:]
                    )
                else:
                    nc.vector.tensor_copy(yo[:], psy[lt][:])
                    nc.scalar.dma_start(
                        y_ap[:, lt, nh2 * 512:(nh2 + 1) * 512], yo[:]
                    )


_NC_CACHE = {}


def _build():
    if "nc" in _NC_CACHE:
        return _NC_CACHE["nc"]
    nc = bacc.Bacc("TRN2", target_bir_lowering=False, debug=False, num_devices=8)
    h_d = nc.dram_tensor("h", [L_CHUNK, H], F32, kind="ExternalInput")
    c1_d = nc.dram_tensor("c1", [L_CHUNK, HH], F32, kind="ExternalInput")
    s1_d = nc.dram_tensor("s1", [L_CHUNK, HH], F32, kind="ExternalInput")
    wq_d = nc.dram_tensor("wq", [H, H], F32, kind="ExternalInput")
    y_d = nc.dram_tensor("y", [L_CHUNK, H], F32, kind="ExternalOutput")
    with tile.TileContext(nc) as tc:
        _emit_once(nc, tc, h_d, c1_d, s1_d, wq_d, y_d)
    nc.compile()
    _NC_CACHE["nc"] = nc
    return nc


def kernel(hidden_states, W_q, cos, sin):
    hs = np.asarray(hidden_states, dtype=np.float32)
    wq = np.ascontiguousarray(np.asarray(W_q, dtype=np.float32))
    cos = np.asarray(cos, dtype=np.float32)
    sin = np.asarray(sin, dtype=np.float32)
    in_maps = []
    for core in range(8):
        b, i = core // 4, core % 4
        sl = slice(i * L_CHUNK, (i + 1) * L_CHUNK)
        in_maps.append({
            "h": np.ascontiguousarray(hs[b, sl]),
            "c1": np.ascontiguousarray(cos[sl, :HH]),
            "s1": np.ascontiguousarray(sin[sl, :HH]),
            "wq": wq,
        })

    nc = _build()
    res = run_bass_kernel_spmd(nc, in_maps, core_ids=list(range(8)))

    out = np.empty((B, L, H), dtype=np.float32)
    for core, r in enumerate(res.results):
        b, i = core // 4, core % 4
        out[b, i * L_CHUNK:(i + 1) * L_CHUNK] = r["y"]
    return out


# revision 40
# speedup vs baseline: 1.0298x; 1.0298x over previous
"""Trainium2 Bass kernel for nn_DenseAttentionOneHead (B=2, L=4096, H=1024).

Reference math:
    h   = hidden * cos + rotate_half(hidden) * sin      (RoPE)
    q   = h @ W_q.T
    out = (q @ h^T) @ h                                 (no softmax)

With no softmax the L x L score matrix factorizes away, and W_q folds in:
    out[b] = h[b] @ M[b],  M[b] = W_q^T G[b],  G[b] = h[b].T @ h[b]  (H x H)

Sharding (8 NeuronCores): cores 0-3 own batch 0's four 1024-row L-chunks,
cores 4-7 batch 1. Each core computes a partial G over its chunk. Instead
of a 4MB fp32 AllReduce of G (cost-model ~225us), the cross-core exchange
is a bf16 ReduceScatter of G (0.5MB out, ~28us) -> each core computes its
256-column slice of M = W_q^T G -> bf16 AllGather of M (2MB out, ~67us).
G is symmetric, so a row-shard of G is a column-shard: the bounce buffers
are laid out shard-major ([rank][k][col-within-shard]) so both the
RS output and the AG output arrive in exactly the layout the next matmul
wants -- no transposes or reshuffles on the critical path.

Engine plan per core: DVE does RoPE tiles 0-6 (+ W/M dequants), Pool does
RoPE tile 7; PE pipelines G's first column-half into 8 persistent PSUM
accumulators while RoPE streams, drains them interleaved with the second
column-half pass, does the h^T transposes (4 per PSUM bank, one wide copy)
under the RS, then the M-slice and y matmuls. No-reader self-matmuls keep
the PE p-state ramped through both collective windows (the cost model
prices post-idle matmuls up to 3.7x slower). ACT/DVE split the PSUM->SBUF
copies; the two HWDGE rings split the traffic (h/G-out/M-in-write/y-even
on sync, cos/sin/W/G-slice/M-fetch/y-odd on scalar).
"""

import os

import numpy as np

import jax

try:
    _cache_dir = os.path.join(os.path.expanduser("~"), ".cache", "bass_kernel_jax")
    os.makedirs(_cache_dir, exist_ok=True)
    jax.config.update("jax_compilation_cache_dir", _cache_dir)
    jax.config.update("jax_persistent_cache_min_compile_time_secs", 1.0)
except Exception:
    pass

import concourse.bacc as bacc
import concourse.mybir as mybir
import concourse.tile as tile
from concourse import masks
from concourse.bass_utils import run_bass_kernel_spmd

F32 = mybir.dt.float32
F32R = mybir.dt.float32r
BF16 = mybir.dt.bfloat16

B, L, H = 2, 4096, 1024
L_CHUNK = 1024
HH = H // 2
NT = L_CHUNK // 128   # 8 row tiles per chunk
MT = H // 128         # 8 column tiles
NSH = H // 4          # 256: columns per RS shard
GROUPS = [[0, 1, 2, 3], [4, 5, 6, 7]]
NACC = 8              # PSUM bank ring ("ps" tag)
WARM1 = 124           # PE keep-warm matmuls during the ReduceScatter
WARM2 = 359           # PE keep-warm matmuls during the AllGather


def _emit_once(nc, tc, h_d, c1_d, s1_d, wq_d, y_d):
    h_ap = h_d.ap().rearrange("(t p) c -> p t c", p=128)
    c1_ap = c1_d.ap().rearrange("(t p) c -> p t c", p=128)
    s1_ap = s1_d.ap().rearrange("(t p) c -> p t c", p=128)
    wq_ap = wq_d.ap().rearrange("(t p) c -> p t c", p=128)
    y_ap = y_d.ap().rearrange("(t p) c -> p t c", p=128)

    with (
        tc.tile_pool(name="persist", bufs=1) as persist,
        tc.tile_pool(name="stream", bufs=1) as stream,
        tc.tile_pool(name="pacc", bufs=NACC, space="PSUM") as pacc,
        tc.tile_pool(name="dram", bufs=1, space="DRAM") as dram,
    ):
        hr = persist.tile([128, NT, H], F32R, name="hr")
        hrt = persist.tile([128, MT, L_CHUNK], F32R, name="hrt")
        wq_b = persist.tile([128, MT, H], BF16, name="wq_b")
        mr = persist.tile([128, MT, H], F32R, name="mr")
        gsl = persist.tile([128, MT, NSH], BF16, name="gsl")

        # DRAM bounce buffers, shard-major layout:
        #   g_in  flat = rank*262144 + k*256 + nw   <->  G_part[k, 256*rank+nw]
        #   g_rs  flat = k*256 + nw                 <->  G[k, 256*myrank+nw]
        #   m_in  flat = hcol*256 + nw              <->  M[hcol, 256*myrank+nw]
        #   m_out flat = rank*262144 + h*256 + nw   <->  M[h, 256*rank+nw]
        g_in = dram.tile([128, MT * H], BF16, name="g_in")
        g_rs = dram.tile([32, MT * H], BF16, name="g_rs")
        m_in = dram.tile([32, MT * H], BF16, name="m_in")
        m_out = dram.tile([128, MT * H], BF16, name="m_out")
        gv = g_in[:].rearrange("(r kp) (kq nw) -> kp kq r nw", r=4, kq=32)
        # one-DMA fetch views: [pp, kt, ...] with k = kt*128 + pp
        rv = (
            g_rs[:]
            .rearrange("p (kq nw) -> (p kq) nw", kq=32)
            .rearrange("(kt pp) nw -> pp kt nw", pp=128)
        )
        mv = (
            m_in[:]
            .rearrange("p (kq nw) -> (p kq) nw", kq=32)
            .rearrange("(mt pp) nw -> pp mt nw", pp=128)
        )
        ov = m_out[:].rearrange("(r kp) (kq nw) -> kp kq r nw", r=4, kq=32)

        # persistent PSUM accumulators for G[:, 0:512], all 8 row bands
        gacc = [
            pacc.tile([128, 512], F32, name=f"gacc{i}", tag="ps")
            for i in range(MT)
        ]

        # ---- RoPE + pipelined G first-half. Tiles 0-6 stream on DVE;
        # tile 7 runs on the otherwise-idle Pool engine so the two engines
        # finish together. ----
        for t in range(NT):
            ht = stream.tile([128, H], F32, name="ht", tag="ld1024", bufs=5)
            ct = stream.tile([128, HH], F32, name="ct", tag="ldc", bufs=5)
            st = stream.tile([128, HH], F32, name="st", tag="lds", bufs=5)
            nc.sync.dma_start(ht[:], h_ap[:, t, :])
            nc.scalar.dma_start(ct[:], c1_ap[:, t, :])
            nc.scalar.dma_start(st[:], s1_ap[:, t, :])
            h1 = ht[:, 0:HH]
            h2 = ht[:, HH:H]
            tag = "tmp" if t < 7 else "tmpp"
            m1 = stream.tile([128, HH], F32, name="m1", tag=tag, bufs=4)
            m2 = stream.tile([128, HH], F32, name="m2", tag=tag, bufs=4)
            m3 = stream.tile([128, HH], F32, name="m3", tag=tag, bufs=4)
            m4 = stream.tile([128, HH], F32, name="m4", tag=tag, bufs=4)
            eng = nc.vector if t < 7 else nc.gpsimd
            eng.tensor_mul(m1[:], h1, ct[:])
            eng.tensor_mul(m2[:], h2, st[:])
            eng.tensor_sub(hr[:, t, 0:HH], m1[:], m2[:])
            eng.tensor_mul(m3[:], h2, ct[:])
            eng.tensor_mul(m4[:], h1, st[:])
            eng.tensor_add(hr[:, t, HH:H], m3[:], m4[:])
            # G[:, 0:512] accumulation rides the RoPE stream
            for mt in range(MT):
                nc.tensor.matmul(
                    gacc[mt][:],
                    hr[:, t, mt * 128:(mt + 1) * 128],
                    hr[:, t, 0:512],
                    start=(t == 0),
                    stop=(t == NT - 1),
                )

        ident_f = stream.tile([128, 128], F32, name="ident_f", tag="identf")
        masks.make_identity(nc, ident_f[:])
        ident = stream.tile([128, 128], F32R, name="ident", tag="ident")
        nc.vector.tensor_copy(ident[:], ident_f[:])

        # W_q load (scalar ring, behind c/s) + bf16 pack on DVE (idle
        # post-RoPE; keeps the ACT queue clear for the G pack copies)
        for mt in range(MT):
            wt = stream.tile([128, H], F32, name="wt", tag="ldw", bufs=2)
            nc.scalar.dma_start(wt[:], wq_ap[:, mt, :])
            nc.vector.tensor_copy(wq_b[:, mt, :], wt[:])

        # drain the pipelined accumulators while computing G cols 512:1024;
        # both halves of a row band pack into one [128,1024] tile -> one DMA
        # (the shard-major dst AP merges to 3 dims for full-width rows).
        for mt in range(MT):
            gb2 = stream.tile([128, H], BF16, name="gb2", tag="gb", bufs=3)
            nc.scalar.copy(gb2[:, 0:512], gacc[mt][:])
            ps = pacc.tile([128, 512], F32, name="ps1", tag="ps")
            for kt in range(NT):
                nc.tensor.matmul(
                    ps[:],
                    hr[:, kt, mt * 128:(mt + 1) * 128],
                    hr[:, kt, 512:1024],
                    start=(kt == 0),
                    stop=(kt == NT - 1),
                )
            nc.scalar.copy(gb2[:, 512:1024], ps[:])
            nc.sync.dma_start(gv[4 * mt:4 * (mt + 1), :, :, :], gb2[:])

        # ReduceScatter of partial G (bf16): each core gets its 256 columns
        nc.gpsimd.collective_compute(
            "ReduceScatter",
            mybir.AluOpType.add,
            replica_groups=GROUPS,
            ins=[g_in[:]],
            outs=[g_rs[:]],
        )

        # transposes for the y-phase stationary h^T (PE idles during RS).
        # 4 transposes share one PSUM bank + one wide ACT copy, so the
        # PE<->ACT semaphore round-trip amortizes 4x.
        for mt in range(MT):
            for ta in range(0, NT, 4):
                pstb = pacc.tile([128, 512], F32R, name="pstb", tag="ps")
                for j in range(4):
                    nc.tensor.transpose(
                        pstb[:, j * 128:(j + 1) * 128],
                        hr[:, ta + j, mt * 128:(mt + 1) * 128],
                        ident[:],
                    )
                nc.vector.tensor_copy(
                    hrt[:, mt, ta * 128:(ta + 4) * 128], pstb[:]
                )

        # self-matmuls with no readers keep the PE p-state ramped through the
        # RS wait (cost model prices post-idle matmuls up to 3.7x slower)
        js = pacc.tile([128, 512], F32, name="js", tag="ps")
        for _ in range(WARM1):
            nc.tensor.matmul(
                js[:], hr[:, 0, 0:128], hr[:, 0, 0:512],
                start=True, stop=True, skip_group_check=True,
            )

        # fetch my G column slice (already [k, nw] matmul layout), one DMA
        nc.scalar.dma_start(gsl[:, :, :], rv[:, :, :])

        # M slice = W_q^T G[:, mycols]  (bf16 matmuls, fp32 PSUM)
        mqall = persist.tile([128, MT, NSH], BF16, name="mqall")
        for mt in range(MT):
            ps = pacc.tile([128, NSH], F32, name="psm", tag="ps")
            for dk in range(MT):
                nc.tensor.matmul(
                    ps[:],
                    wq_b[:, dk, mt * 128:(mt + 1) * 128],
                    gsl[:, dk, :],
                    start=(dk == 0),
                    stop=(dk == MT - 1),
                )
            nc.scalar.copy(mqall[:, mt, :], ps[:])
        nc.sync.dma_start(mv[:, :, :], mqall[:, :, :])

        # AllGather of M slices (bf16)
        nc.gpsimd.collective_compute(
            "AllGather",
            mybir.AluOpType.bypass,
            replica_groups=GROUPS,
            ins=[m_in[:]],
            outs=[m_out[:]],
        )

        # keep the PE warm through the AllGather idle window
        js2 = pacc.tile([128, 512], F32, name="js2", tag="ps")
        for _ in range(WARM2):
            nc.tensor.matmul(
                js2[:], hr[:, 0, 0:128], hr[:, 0, 0:512],
                start=True, stop=True, skip_group_check=True,
            )

        # fetch + dequant M per-tile (DVE), pipelined with the y matmuls
        for ht in range(MT):
            mf = stream.tile([128, H], BF16, name="mf", tag="mf", bufs=3)
            nc.scalar.dma_start(mf[:], ov[4 * ht:4 * (ht + 1), :, :, :])
            nc.vector.tensor_copy(mr[:, ht, :], mf[:])

        # y = h @ M. First half k-outer (starts on the first dequanted M
        # tile); second half lt-major so the 8 accumulator stops stagger and
        # the output copies/DMAs drain under the remaining matmuls instead
        # of all queueing after the last one.
        def y_out(lt, nh2, ps):
            yo = stream.tile([128, 512], F32, name="yo", tag="yo", bufs=4)
            if lt % 2 == 0:
                nc.scalar.copy(yo[:], ps[:])
                nc.sync.dma_start(
                    y_ap[:, lt, nh2 * 512:(nh2 + 1) * 512], yo[:]
                )
            else:
                nc.vector.tensor_copy(yo[:], ps[:])
                nc.scalar.dma_start(
                    y_ap[:, lt, nh2 * 512:(nh2 + 1) * 512], yo[:]
                )

        psy = [
            pacc.tile([128, 512], F32, name=f"psy{lt}", tag="ps")
            for lt in range(NT)
        ]
        for ht in range(MT):
            for lt in range(NT):
                nc.tensor.matmul(
                    psy[lt][:],
                    hrt[:, ht, lt * 128:(lt + 1) * 128],
                    mr[:, ht, 0:512],
                    start=(ht == 0),
                    stop=(ht == MT - 1),
                )
        for lt in range(NT):
            y_out(lt, 0, psy[lt])
        for lt in range(NT):
            ps2 = pacc.tile([128, 512], F32, name="psy2", tag="ps")
            for ht in range(MT):
                nc.tensor.matmul(
                    ps2[:],
                    hrt[:, ht, lt * 128:(lt + 1) * 128],
                    mr[:, ht, 512:1024],
                    start=(ht == 0),
                    stop=(ht == MT - 1),
                )
            y_out(lt, 1, ps2)


_NC_CACHE = {}


def _build():
    if "nc" in _NC_CACHE:
        return _NC_CACHE["nc"]
    nc = bacc.Bacc("TRN2", target_bir_lowering=False, debug=False, num_devices=8)
    h_d = nc.dram_tensor("h", [L_CHUNK, H], F32, kind="ExternalInput")
    c1_d = nc.dram_tensor("c1", [L_CHUNK, HH], F32, kind="ExternalInput")
    s1_d = nc.dram_tensor("s1", [L_CHUNK, HH], F32, kind="ExternalInput")
    wq_d = nc.dram_tensor("wq", [H, H], F32, kind="ExternalInput")
    y_d = nc.dram_tensor("y", [L_CHUNK, H], F32, kind="ExternalOutput")
    with tile.TileContext(nc) as tc:
        _emit_once(nc, tc, h_d, c1_d, s1_d, wq_d, y_d)
    nc.compile()
    _NC_CACHE["nc"] = nc
    return nc


def kernel(hidden_states, W_q, cos, sin):
    hs = np.asarray(hidden_states, dtype=np.float32)
    wq = np.ascontiguousarray(np.asarray(W_q, dtype=np.float32))
    cos = np.asarray(cos, dtype=np.float32)
    sin = np.asarray(sin, dtype=np.float32)
    in_maps = []
    for core in range(8):
        b, i = core // 4, core % 4
        sl = slice(i * L_CHUNK, (i + 1) * L_CHUNK)
        in_maps.append({
            "h": np.ascontiguousarray(hs[b, sl]),
            "c1": np.ascontiguousarray(cos[sl, :HH]),
            "s1": np.ascontiguousarray(sin[sl, :HH]),
            "wq": wq,
        })

    nc = _build()
    res = run_bass_kernel_spmd(nc, in_maps, core_ids=list(range(8)))

    out = np.empty((B, L, H), dtype=np.float32)
    for core, r in enumerate(res.results):
        b, i = core // 4, core % 4
        out[b, i * L_CHUNK:(i + 1) * L_CHUNK] = r["y"]
    return out
